# revision 1
# baseline (speedup 1.0000x reference)
"""CalderaLinear fused kernel for 8 Trainium2 NeuronCores.

Math (reference): y = x @ Q^T + (x @ R^T) @ L^T + bias, with Q/L/R groupwise
int-dequantized (codes 0..15, group size 128).

Strategy:
  * Column-parallel over d_out: core c owns out-features [c*512, (c+1)*512).
  * On each core, first build W_c = Q_c^T + R^T @ L_c^T  ([d_in, 512]) on-chip:
    R and L^T are dequantized with DVE multiplies (codes and pre-broadcast
    scales arrive as exact bf16), R^T L^T accumulates on the PE into PSUM, and
    dequantized Q^T is added during PSUM eviction into the resident W tile.
  * Then y_c = x @ W_c + bias_c: x streams through as 128x512 pre-tiled
    blocks (host-side retiling gives one contiguous DMA per tile), W_c stays
    SBUF-resident, PSUM accumulates over the 32 k-tiles, bias is fused into
    the PSUM eviction.
  * All W-build operands are packed host-side into one contiguous
    [128, 51200] blob so the build phase streams in as a handful of large
    DMAs (the per-tensor version paid ~2.5us of cold-queue latency per small
    DMA at kernel start).
  * Host side only reshapes/transposes/casts and concatenates the 8 output
    shards: all dequant + matmul math runs on the NeuronCores.

Compute dtype is bf16 (codes 0..15 are exact; rel-err ~3e-3 vs fp32
reference, dominated by bf16 rounding of x and W). Set CALDERA_DTYPE=float32r
for the reduced-precision-fp32 PE mode (~2e-4 rel-err, ~15% slower).
"""

import os
import numpy as np
import ml_dtypes

P = 128
D_IN = 4096
D_OUT = 4096
TOK = 8192
RANK = 256
NCORES = 8
OC = D_OUT // NCORES      # 512 out features per core
KT = D_IN // P            # 32 contraction tiles
MS = 512                  # token slab
NS = TOK // MS            # 16 slabs
SUB = MS // P             # 4 psum sub-tiles per slab
KG = D_IN // 128          # 32 scale groups along d_in
RG = RANK // 128          # 2 scale groups along rank

# ---- W-build blob layout (columns, per partition), consumption-ordered ----
# [ ltv_j0 | lstb_j0 | ltv_j1 | lstb_j1 ]                    header: 2048
# then per R-chunk ch (8 chunks of 512 cols, covering W k-tiles 4ch..4ch+3):
#   [ rv_j0 | rs_j0 | rv_j1 | rs_j1 ]                        2048
#   [ qc_{4ch} | qb_{4ch} | qc_{4ch+1} | qb_{4ch+1} ]        2048
#   [ qc_{4ch+2} | qb_{4ch+2} | qc_{4ch+3} | qb_{4ch+3} ]    2048
RCH = 8
RCW = D_IN // RCH         # 512 R columns per chunk
HDR = RG * 2 * OC         # 2048
SEG = 3 * 2048            # per-chunk segment
WBCOLS = HDR + RCH * SEG  # 51200


def _rv_off(j, ch):
    return HDR + ch * SEG + j * 2 * RCW


def _rs_off(j, ch):
    return _rv_off(j, ch) + RCW


def _qc_off(k):
    return HDR + (k // 4) * SEG + 2048 + (k % 4) * 2 * OC


def _qb_off(k):
    return _qc_off(k) + OC

_module_cache = {}
last_result = None


def _build_module(dt_name):
    import concourse.mybir as mybir
    import concourse.tile as tile
    from concourse import bacc

    use_f32r = dt_name == "float32r"
    dt_c = getattr(mybir.dt, dt_name)
    f32 = mybir.dt.float32

    def mm(ap):
        return ap

    nc = bacc.Bacc(None, target_bir_lowering=False, debug=False)
    xt_d = nc.dram_tensor("xt", (NS, KT, P, MS), dt_c, kind="ExternalInput")
    wb_d = nc.dram_tensor("wb", (P, WBCOLS), dt_c, kind="ExternalInput")
    bias_d = nc.dram_tensor("biasv", (P, OC), f32, kind="ExternalInput")
    y_d = nc.dram_tensor("y", (TOK, OC), f32, kind="ExternalOutput")

    with tile.TileContext(nc) as tc:
        with (
            tc.tile_pool(name="const", bufs=1) as const,
            tc.tile_pool(name="wpool", bufs=1) as wpool,
            tc.tile_pool(name="xpool", bufs=16) as xpool,
            tc.tile_pool(name="qpool", bufs=4) as qpool,
            tc.tile_pool(name="ypool", bufs=8) as ypool,
            tc.tile_pool(name="ppool", bufs=6, space="PSUM") as ppool,
            tc.tile_pool(name="wbpool", bufs=2, space="PSUM") as wbpool,
        ):
            # In f32r mode only the header+R pieces stay SBUF-resident
            # (budget); Q pieces stream through qpool inside build_w instead.
            rseg = 2048 if use_f32r else SEG
            WB = const.tile([P, HDR + RCH * rseg], dt_c)
            bias_t = const.tile([P, OC], f32)

            def ltv(j):
                return WB[:, j * 2 * OC:j * 2 * OC + OC]

            def lst(j):
                return WB[:, j * 2 * OC + OC:(j + 1) * 2 * OC]

            def rv(j, ch):
                o = HDR + ch * rseg + j * 2 * RCW
                return WB[:, o:o + RCW]

            def rs(j, ch):
                o = HDR + ch * rseg + j * 2 * RCW + RCW
                return WB[:, o:o + RCW]

            def qc(k):
                return WB[:, _qc_off(k):_qc_off(k) + OC]

            def qb(k):
                return WB[:, _qb_off(k):_qb_off(k) + OC]

            # blob streams in consumption order as 0.5 MB pieces
            nc.sync.dma_start(WB[:, 0:HDR], wb_d[:, 0:HDR])
            for ch in range(RCH):
                for po in range(0, rseg, 2048):
                    nc.sync.dma_start(
                        WB[:, HDR + ch * rseg + po:HDR + ch * rseg + po + 2048],
                        wb_d[:, HDR + ch * SEG + po:HDR + ch * SEG + po + 2048],
                    )
            nc.sync.dma_start(bias_t[:], bias_d[:])

            # ---- dequantize L^T and R (codes x pre-broadcast scales).
            # R dequantizes in place over its code slice in the blob.
            LdT = const.tile([P, RG, OC], dt_c)
            for j in range(RG):
                nc.vector.tensor_mul(LdT[:, j, :], ltv(j), lst(j))

            def dequant_r(ch):
                # deferred per-chunk so the in-order DVE stream never blocks
                # the first W evictions on late R-chunk DMAs
                for j in range(RG):
                    nc.vector.tensor_mul(rv(j, ch), rv(j, ch), rs(j, ch))

            def rd(j, k):
                # dequantized R columns for W k-tile k (128 cols)
                base = HDR + (k // 4) * rseg + j * 2 * RCW + (k % 4) * P
                return WB[:, base:base + P]

            dequant_r(0)

            # ---- W_c = R^T @ L^T + Q^T, built one k-tile at a time.
            # The build is interleaved into slab 0's k-loop two tiles ahead
            # (build W[k+2] while slab 0 multiplies with W[k]) so the
            # DVE-bound build chain (~1.25us/k) hides under PE matmul work.
            Wt = wpool.tile([P, KT, OC], dt_c)

            def build_w(k):
                ps = wbpool.tile([P, OC], f32, tag="wb", name=f"wb{k}")
                for j in range(RG):
                    nc.tensor.matmul(
                        ps[:],
                        mm(rd(j, k)),
                        mm(LdT[:, j, :]),
                        start=(j == 0),
                        stop=(j == RG - 1),
                    )
                if use_f32r:
                    qt = qpool.tile([P, 2 * OC], dt_c, tag="qt")
                    nc.sync.dma_start(qt[:], wb_d[:, _qc_off(k):_qc_off(k) + 2 * OC])
                    qc_ap, qb_ap = qt[:, :OC], qt[:, OC:]
                else:
                    qc_ap, qb_ap = qc(k), qb(k)
                qdq = qpool.tile([P, OC], dt_c, tag="qd")
                nc.vector.tensor_mul(qdq[:], qc_ap, qb_ap)
                nc.vector.tensor_add(Wt[:, k, :], ps[:], qdq[:])

            def evict(psums, s):
                for sub in range(SUB):
                    yt = ypool.tile([P, OC], f32, tag="y", name=f"y{s}_{sub}")
                    nc.vector.tensor_add(yt[:], psums[sub][:], bias_t[:])
                    nc.scalar.dma_start(
                        y_d[s * MS + sub * P:s * MS + (sub + 1) * P, :], yt[:]
                    )

            for _k in range(3):
                build_w(_k)
            psums0 = [ppool.tile([P, OC], f32, tag="ps", name=f"ps0_{i}")
                      for i in range(SUB)]
            for k in range(KT):
                xt = xpool.tile([P, MS], dt_c, tag="x", name="xt0")
                nc.scalar.dma_start(xt[:], xt_d[0, k])
                for sub in range(SUB):
                    nc.tensor.matmul(
                        psums0[sub][:], mm(xt[:, sub * P:(sub + 1) * P]),
                        mm(Wt[:, k, :]), start=(k == 0), stop=(k == KT - 1),
                    )
                if k + 3 < KT:
                    if (k + 3) % (KT // RCH) == 0:
                        dequant_r((k + 3) // (KT // RCH))
                    build_w(k + 3)
                # KT//RCH == 4: chunk ch feeds W k-tiles 4ch..4ch+3
            evict(psums0, 0)

            for s in range(1, NS):
                psums = [
                    ppool.tile([P, OC], f32, tag="ps", name=f"ps{s}_{i}")
                    for i in range(SUB)
                ]
                for k in range(KT):
                    xt = xpool.tile([P, MS], dt_c, tag="x")
                    dma_eng = nc.sync if k % 2 == 0 else nc.scalar
                    dma_eng.dma_start(xt[:], xt_d[s, k])
                    for sub in range(SUB):
                        nc.tensor.matmul(
                            psums[sub][:],
                            mm(xt[:, sub * P:(sub + 1) * P]),
                            mm(Wt[:, k, :]),
                            start=(k == 0),
                            stop=(k == KT - 1),
                        )
                evict(psums, s)

    nc.compile()
    return nc


def kernel(x, q_values, q_scales, l_values, l_scales, r_values, r_scales, bias,
           _trace=False):
    from concourse.bass_utils import run_bass_kernel_spmd

    dt_name = os.environ.get("CALDERA_DTYPE", "bfloat16")
    np_in = ml_dtypes.bfloat16 if dt_name == "bfloat16" else np.float32

    if dt_name not in _module_cache:
        _module_cache[dt_name] = _build_module(dt_name)
    nc = _module_cache[dt_name]

    # host-side marshaling (layout + dtype only; all math runs on-device)
    x = np.asarray(x, dtype=np.float32)
    q_values = np.asarray(q_values)
    q_scales = np.asarray(q_scales)
    l_values = np.asarray(l_values)
    l_scales = np.asarray(l_scales)
    r_values = np.asarray(r_values)
    r_scales = np.asarray(r_scales)
    bias = np.asarray(bias)
    # xt[s, k, p, m] = x[s*MS + m, k*P + p]
    xt = np.ascontiguousarray(
        x.reshape(NS, MS, KT, P).transpose(0, 2, 3, 1)
    ).astype(np_in)
    rs_full = np.repeat(np.asarray(r_scales, np.float32), D_IN // KG, axis=1)
    rv_f = np.asarray(r_values, np.float32)

    in_maps = []
    for c in range(NCORES):
        sl = slice(c * OC, (c + 1) * OC)
        qt_c = q_values[sl].T.astype(np.float32)           # [D_IN, OC]
        qst_c = q_scales[sl].T.astype(np.float32)          # [KT, OC]
        ltv_c = l_values[sl].T.astype(np.float32)          # [RANK, OC]
        lst_c = l_scales[sl].T.astype(np.float32)          # [RG, OC]

        pieces = []
        for j in range(RG):
            pieces.append(ltv_c[j * P:(j + 1) * P, :])
            pieces.append(np.broadcast_to(lst_c[j].reshape(1, OC), (P, OC)))
        for ch in range(RCH):
            cs = slice(ch * RCW, (ch + 1) * RCW)
            for j in range(RG):
                pieces.append(rv_f[j * P:(j + 1) * P, cs])
                pieces.append(rs_full[j * P:(j + 1) * P, cs])
            for k in range(4 * ch, 4 * ch + 4):
                pieces.append(qt_c[k * P:(k + 1) * P, :])
                pieces.append(np.broadcast_to(qst_c[k].reshape(1, OC), (P, OC)))
        wb = np.concatenate(pieces, axis=1).astype(np_in)
        assert wb.shape == (P, WBCOLS)

        in_maps.append({
            "xt": xt,
            "wb": wb,
            "biasv": np.ascontiguousarray(
                np.broadcast_to(bias[sl].reshape(1, OC), (P, OC))
            ).astype(np.float32),
        })

    res = run_bass_kernel_spmd(
        nc, in_maps, core_ids=list(range(NCORES)), trace=_trace
    )
    global last_result
    last_result = res
    return np.concatenate([r["y"] for r in res.results], axis=1)



# revision 17
# speedup vs baseline: 1.1166x; 1.1166x over previous
"""CalderaLinear fused kernel for 8 Trainium2 NeuronCores — fp8 DoubleRow.

Math (reference): y = x @ Q^T + (x @ R^T) @ L^T + bias, with Q/L/R groupwise
int-dequantized (codes 0..15, group size 128).

Strategy (v2, fp8):
  * Column-parallel over d_out: core c owns out-features [c*512, (c+1)*512).
  * W_c = Q_c^T + R^T L_c^T ([d_in, 512]) has column means ~3600 (the R^T L^T
    product of non-negative codes) while the fluctuation around the mean is
    only ~270 rms.  fp8 e4m3's 3-bit mantissa on raw W gives ~2.5e-2 rel
    error (fails the 2e-2 gate), but on the *centered* W it gives ~4e-3.
    So the kernel computes, all on device:
        m_r   = mean_i r_deq[r, i]                    (DVE reduce)
        What  = Q^T/16 + (R - 1 m)^T (L/16)^T          (PE + DVE, cast e4m3)
        mu_o  = 16 * m @ (L/16)^T                      (PE, rank-1 weights)
        S_t   = sum_i x[t, i]                          (DVE reduce over bf16 x)
        y     = (16x)_fp8 @ What_fp8 + S * mu + bias   (PE DoubleRow + DVE/ACT)
    The rank-1 S*mu term restores the removed mean exactly; the fp8 rounding
    only ever touches the small fluctuating part.  Simulated rel_l2 ~4e-3.
  * The main matmul runs in MatmulPerfMode.DoubleRow: both operands e4m3,
    3D APs [128, 2, free] carrying two contraction planes per partition,
    contraction 256 per matmul -> half the matmul count of bf16.
  * S is data-parallel: core c reduces its own 1024 token rows (raw-layout
    bf16 x shard), then a 4 KB AllGather shares all 8192 sums.  Evictions
    are split: psum+bias -> SBUF immediately (frees PSUM), the +S*mu
    correction + store lag one slab so the collective latency hides.
  * x streams as pre-tiled fp8 [128, 2, 512] blocks; W stays SBUF-resident
    fp8; PSUM accumulates over the 16 doubled k-tiles; bias fused into
    eviction.  Host side only reshapes/transposes/casts/scales-by-2^±4 and
    concatenates the 8 output shards: all dequant + matmul + reduction math
    runs on the NeuronCores.
"""

import numpy as np
import ml_dtypes

P = 128
D_IN = 4096
D_OUT = 4096
TOK = 8192
RANK = 256
NCORES = 8
OC = D_OUT // NCORES      # 512 out features per core
KT = D_IN // P            # 32 contraction tiles (bf16 build granularity)
KKT = KT // 2             # 16 doubled contraction tiles (fp8 DoubleRow)
MS = 512                  # token slab
NS = TOK // MS            # 16 slabs
SUB = MS // P             # 4 psum sub-tiles per slab
RG = RANK // 128          # 2 rank tiles
RCH = 8                   # R chunks along d_in
RCW = D_IN // RCH         # 512 R columns per chunk
SHTOK = TOK // NCORES     # 1024 tokens per core for the S reduction
SHT = SHTOK // P          # 8 row-tiles of the S shard

_module_cache = {}
last_result = None


def _build_module():
    import concourse.mybir as mybir
    import concourse.tile as tile
    from concourse import bacc

    bf = mybir.dt.bfloat16
    f8 = mybir.dt.float8e4
    f32 = mybir.dt.float32
    AX = mybir.AxisListType
    DR = mybir.MatmulPerfMode.DoubleRow
    COPY = mybir.ActivationFunctionType.Copy

    nc = bacc.Bacc(None, target_bir_lowering=False, debug=False,
                   num_devices=NCORES)
    xt_d = nc.dram_tensor("xt", (NS, KKT, P, 2, MS), f8, kind="ExternalInput")
    wbl_d = nc.dram_tensor("wbl", (P, RG, 2, OC), bf, kind="ExternalInput")
    wbr_d = nc.dram_tensor("wbr", (P, RCH, RG, 2, RCW), bf, kind="ExternalInput")
    wbq_d = nc.dram_tensor("wbq", (P, KT, 2, OC), bf, kind="ExternalInput")
    XBH = D_IN // 2
    xb_d = nc.dram_tensor("xb", (SHT, P, 2, XBH), bf, kind="ExternalInput")
    bias_d = nc.dram_tensor("biasv", (P, OC), f32, kind="ExternalInput")
    y_d = nc.dram_tensor("y", (TOK, OC), f32, kind="ExternalOutput")

    with tile.TileContext(nc) as tc:
        with (
            tc.tile_pool(name="const", bufs=1) as const,
            tc.tile_pool(name="wpool", bufs=1) as wpool,
            tc.tile_pool(name="xpool", bufs=16) as xpool,
            tc.tile_pool(name="xbpool", bufs=8) as xbpool,
            tc.tile_pool(name="qpool", bufs=4) as qpool,
            tc.tile_pool(name="ypool", bufs=14) as ypool,
            tc.tile_pool(name="cpool", bufs=4) as cpool,
            tc.tile_pool(name="ppool", bufs=6, space="PSUM") as ppool,
            tc.tile_pool(name="wbpool", bufs=2, space="PSUM") as wbpool,
            tc.tile_pool(name="dpool", bufs=1, space="DRAM") as dpool,
        ):
            sin_d = dpool.tile([P, SHT], f32, name="sin")
            sout_d = dpool.tile([NCORES, P, SHT], f32, name="sout",
                                addr_space="Shared")
            WBL = const.tile([P, RG, 2, OC], bf)
            WBR = const.tile([P, RCH, RG, 2, RCW], bf)
            bias_t = const.tile([P, OC], f32)
            LdT = const.tile([P, RG, OC], bf)
            Wt = wpool.tile([P, KKT, 2, OC], f8)
            mneg = const.tile([P, RG], f32)     # -mean_i r_deq (per rank row)
            m16 = const.tile([P, RG], bf)       # +16/16... = +m (bf16, mu lhsT)
            ones1 = const.tile([1, P], bf)
            murow = const.tile([1, OC], bf)
            mub = const.tile([P, OC], f32)      # broadcast 16*mu/16 = mu
            S_sb = const.tile([P, NS * SUB], f32)
            Sown = const.tile([P, SHT], f32)
            Spart = const.tile([P, SHT, 2], f32)

            # ---- phase-0 DMAs.  sync: weights blob; gpsimd: bf16 x shard.
            nc.sync.dma_start(bias_t[:], bias_d[:])
            nc.sync.dma_start(WBL[:], wbl_d[:])
            for ch in range(RCH):
                nc.sync.dma_start(WBR[:, ch], wbr_d[:, ch])
            xbt = []
            for j in range(SHT):
                for h in range(2):
                    t = xbpool.tile([P, XBH], bf, tag="xb", name=f"xb{j}_{h}")
                    nc.gpsimd.dma_start(t[:], xb_d[j, :, h])
                    xbt.append(t)

            # ---- dequantize L^T (codes x pre-broadcast scales/16)
            for j in range(RG):
                nc.vector.tensor_mul(LdT[:, j, :], WBL[:, j, 0, :],
                                     WBL[:, j, 1, :])
            nc.vector.memset(ones1[:], 1.0)

            # ---- dequantize R in place, compute -mean, center
            for ch in range(RCH):
                for j in range(RG):
                    nc.vector.tensor_mul(WBR[:, ch, j, 0, :],
                                         WBR[:, ch, j, 0, :],
                                         WBR[:, ch, j, 1, :])
            for j in range(RG):
                nc.vector.reduce_sum(mneg[:, j:j + 1], WBR[:, :, j, 0, :],
                                     axis=AX.XY, negate=True)
            nc.vector.tensor_scalar_mul(m16[:], mneg[:], -1.0 / D_IN)
            nc.vector.tensor_scalar_mul(mneg[:], mneg[:], 1.0 / D_IN)
            for ch in range(RCH):
                for j in range(RG):
                    nc.vector.tensor_scalar_add(WBR[:, ch, j, 0, :],
                                                WBR[:, ch, j, 0, :],
                                                mneg[:, j:j + 1])

            # ---- mu = 16 * m @ (L/16)^T, broadcast to all 128 partitions
            mu_ps = wbpool.tile([P, OC], f32, tag="wb", name="mu1")
            for j in range(RG):
                nc.tensor.matmul(mu_ps[0:1, :], m16[:, j:j + 1], LdT[:, j, :],
                                 start=(j == 0), stop=(j == RG - 1))
            nc.vector.tensor_copy(murow[:], mu_ps[0:1, :])
            mub_ps = wbpool.tile([P, OC], f32, tag="wb", name="mu2")
            nc.tensor.matmul(mub_ps[:], ones1[:], murow[:], start=True,
                             stop=True)
            nc.vector.tensor_scalar_mul(mub[:], mub_ps[:], 16.0)

            # ---- W build: What k-tile k = R_hat^T(L/16)^T + Q^T/16 -> e4m3
            def rd(j, k):
                return WBR[:, k // 4, j, 0, (k % 4) * P:(k % 4) * P + P]

            def build_w(k):
                ps = wbpool.tile([P, OC], f32, tag="wb", name=f"wb{k}")
                for j in range(RG):
                    nc.tensor.matmul(ps[:], rd(j, k), LdT[:, j, :],
                                     start=(j == 0), stop=(j == RG - 1))
                qt = qpool.tile([P, 2, OC], bf, tag="qt")
                nc.sync.dma_start(qt[:], wbq_d[:, k])
                nc.vector.tensor_mul(qt[:, 0, :], qt[:, 0, :], qt[:, 1, :])
                nc.vector.tensor_add(Wt[:, k // 2, k % 2, :], ps[:],
                                     qt[:, 0, :])

            psums = {}

            def slab_mms(s, dma_engines):
                psums[s] = [ppool.tile([P, OC], f32, tag="ps",
                                       name=f"ps{s}_{i}") for i in range(SUB)]
                for kk in range(KKT):
                    xt = xpool.tile([P, 2, MS], f8, tag="x")
                    dma_engines[kk % len(dma_engines)].dma_start(
                        xt[:], xt_d[s, kk])
                    for sub in range(SUB):
                        nc.tensor.matmul(
                            psums[s][sub][:],
                            xt[:, :, sub * P:(sub + 1) * P],
                            Wt[:, kk, :, :],
                            start=(kk == 0), stop=(kk == KKT - 1),
                            perf_mode=DR,
                        )
                    yield kk

            ytiles = {}

            def evict_a(s):
                # psum + bias -> SBUF, frees the psum bank
                ytiles[s] = []
                for sub in range(SUB):
                    yt = ypool.tile([P, OC], f32, tag="y", name=f"y{s}_{sub}")
                    nc.vector.tensor_add(yt[:], psums[s][sub][:], bias_t[:])
                    ytiles[s].append(yt)

            def evict_b(s):
                # + S*mu (ACT outer product), then store
                for sub in range(SUB):
                    col = s * SUB + sub
                    corr = cpool.tile([P, OC], f32, tag="c")
                    nc.scalar.activation(corr[:], mub[:], COPY,
                                         scale=S_sb[:, col:col + 1])
                    yt = ytiles[s][sub]
                    nc.vector.tensor_add(yt[:], yt[:], corr[:])
                    nc.scalar.dma_start(
                        y_d[s * MS + sub * P:s * MS + (sub + 1) * P, :], yt[:])

            # ---- slab 0 with the W build interleaved two tiles ahead
            for k in range(4):
                build_w(k)
            for kk in slab_mms(0, [nc.scalar]):
                if kk < KKT - 2:
                    build_w(2 * kk + 4)
                    build_w(2 * kk + 5)

            evict_a(0)

            # ---- S shard: reduce own 1024 bf16 token rows, AllGather 4 KB
            for i in range(2 * SHT):
                h = i % 2
                nc.vector.reduce_sum(Spart[:, i // 2, h:h + 1], xbt[i][:],
                                     axis=AX.X)
            nc.vector.reduce_sum(Sown[:], Spart[:], axis=AX.X)
            nc.gpsimd.dma_start(sin_d[:], Sown[:])
            nc.gpsimd.collective_compute(
                "AllGather",
                mybir.AluOpType.bypass,
                replica_groups=[list(range(NCORES))],
                ins=[sin_d.opt()],
                outs=[sout_d.opt()],
            )
            for c in range(NCORES):
                nc.gpsimd.dma_start(S_sb[:, c * SHT:(c + 1) * SHT], sout_d[c])

            for s in range(1, NS):
                for kk in slab_mms(s, [nc.sync, nc.scalar]):
                    pass
                evict_a(s)
                if s >= 2:
                    evict_b(s - 2)
            evict_b(NS - 2)
            evict_b(NS - 1)

    nc.compile()
    return nc


def kernel(x, q_values, q_scales, l_values, l_scales, r_values, r_scales, bias,
           _trace=False):
    from concourse.bass_utils import run_bass_kernel_spmd

    bf16 = ml_dtypes.bfloat16
    e4m3 = ml_dtypes.float8_e4m3

    if "m" not in _module_cache:
        _module_cache["m"] = _build_module()
    nc = _module_cache["m"]

    # host-side marshaling (layout + dtype + power-of-two scaling only)
    x = np.asarray(x, dtype=np.float32)
    q_values = np.asarray(q_values)
    q_scales = np.asarray(q_scales, np.float32)
    l_values = np.asarray(l_values)
    l_scales = np.asarray(l_scales, np.float32)
    r_values = np.asarray(r_values)
    r_scales = np.asarray(r_scales, np.float32)
    bias = np.asarray(bias, np.float32)

    # x*16 as e4m3, tiled [NS, KKT, P, 2, MS]: plane ko holds i = kk*256+ko*128+p
    xs = np.clip(x * 16.0, -240.0, 240.0)
    xt8 = np.ascontiguousarray(
        xs.reshape(NS, MS, KKT, 2, P).transpose(0, 2, 4, 3, 1)
    ).astype(e4m3)
    # bf16 raw-row x for the S reduction, per-core shard below
    xb_all = x.astype(bf16)

    rs_full = np.repeat(r_scales, D_IN // r_scales.shape[1], axis=1)
    rv_f = r_values.astype(np.float32)
    # wbr[p, ch, j, 0, :] = r codes, [.., 1, :] = broadcast r scales
    wbr = np.empty((P, RCH, RG, 2, RCW), np.float32)
    for ch in range(RCH):
        cs = slice(ch * RCW, (ch + 1) * RCW)
        for j in range(RG):
            wbr[:, ch, j, 0, :] = rv_f[j * P:(j + 1) * P, cs]
            wbr[:, ch, j, 1, :] = rs_full[j * P:(j + 1) * P, cs]
    wbr = wbr.astype(bf16)

    in_maps = []
    for c in range(NCORES):
        sl = slice(c * OC, (c + 1) * OC)
        qt_c = q_values[sl].T.astype(np.float32)            # [D_IN, OC]
        qst_c = (q_scales[sl].T / 16.0).astype(np.float32)  # [KT, OC]
        ltv_c = l_values[sl].T.astype(np.float32)           # [RANK, OC]
        lst_c = (l_scales[sl].T / 16.0).astype(np.float32)  # [RG, OC]

        wbl = np.empty((P, RG, 2, OC), np.float32)
        for j in range(RG):
            wbl[:, j, 0, :] = ltv_c[j * P:(j + 1) * P, :]
            wbl[:, j, 1, :] = np.broadcast_to(lst_c[j], (P, OC))
        wbq = np.empty((P, KT, 2, OC), np.float32)
        for k in range(KT):
            wbq[:, k, 0, :] = qt_c[k * P:(k + 1) * P, :]
            wbq[:, k, 1, :] = np.broadcast_to(qst_c[k], (P, OC))

        in_maps.append({
            "xt": xt8,
            "wbl": wbl.astype(bf16),
            "wbr": wbr,
            "wbq": wbq.astype(bf16),
            "xb": np.ascontiguousarray(
                xb_all[c * SHTOK:(c + 1) * SHTOK].reshape(SHT, P, 2, D_IN // 2)),
            "biasv": np.ascontiguousarray(
                np.broadcast_to(bias[sl], (P, OC))).astype(np.float32),
        })

    res = run_bass_kernel_spmd(
        nc, in_maps, core_ids=list(range(NCORES)), trace=_trace
    )
    global last_result
    last_result = res
    return np.concatenate([r["y"] for r in res.results], axis=1)


# revision 27
# speedup vs baseline: 1.3086x; 1.1720x over previous
"""CalderaLinear fused kernel for 8 Trainium2 NeuronCores — fp8 DoubleRow.

Math (reference): y = x @ Q^T + (x @ R^T) @ L^T + bias, with Q/L/R groupwise
int-dequantized (codes 0..15, group size 128).

Strategy (v2, fp8):
  * Column-parallel over d_out: core c owns out-features [c*512, (c+1)*512).
  * W_c = Q_c^T + R^T L_c^T ([d_in, 512]) has column means ~3600 (the R^T L^T
    product of non-negative codes) while the fluctuation around the mean is
    only ~270 rms.  fp8 e4m3's 3-bit mantissa on raw W gives ~2.5e-2 rel
    error (fails the 2e-2 gate), but on the *centered* W it gives ~4e-3.
    So the kernel computes, all on device:
        m_r   = mean_i r_deq[r, i]                    (DVE reduce)
        What  = Q^T/16 + (R - 1 m)^T (L/16)^T          (PE + DVE, cast e4m3)
        mu_o  = 16 * m @ (L/16)^T                      (PE, rank-1 weights)
        S_t   = sum_i x[t, i]                          (DVE reduce over bf16 x)
        y     = (16x)_fp8 @ What_fp8 + S * mu + bias   (PE DoubleRow + DVE/ACT)
    The rank-1 S*mu term restores the removed mean exactly; the fp8 rounding
    only ever touches the small fluctuating part.  Simulated rel_l2 ~4e-3.
  * The main matmul runs in MatmulPerfMode.DoubleRow: both operands e4m3,
    3D APs [128, 2, free] carrying two contraction planes per partition,
    contraction 256 per matmul -> half the matmul count of bf16.
  * S is data-parallel: core c reduces its own 1024 token rows (raw-layout
    bf16 x shard), then a 4 KB AllGather shares all 8192 sums.  Evictions
    are split: psum+bias -> SBUF immediately (frees PSUM), the +S*mu
    correction + store lag one slab so the collective latency hides.
  * x streams as pre-tiled fp8 [128, 2, 512] blocks; W stays SBUF-resident
    fp8; PSUM accumulates over the 16 doubled k-tiles; bias fused into
    eviction.  Host side only reshapes/transposes/casts/scales-by-2^±4 and
    concatenates the 8 output shards: all dequant + matmul + reduction math
    runs on the NeuronCores.
"""

import numpy as np
import ml_dtypes

P = 128
D_IN = 4096
D_OUT = 4096
TOK = 8192
RANK = 256
NCORES = 8
OC = D_OUT // NCORES      # 512 out features per core
KT = D_IN // P            # 32 contraction tiles (bf16 build granularity)
KKT = KT // 2             # 16 doubled contraction tiles (fp8 DoubleRow)
MS = 512                  # token slab
NS = TOK // MS            # 16 slabs
SUB = MS // P             # 4 psum sub-tiles per slab
RG = RANK // 128          # 2 rank tiles
RCH = 8                   # R chunks along d_in
RCW = D_IN // RCH         # 512 R columns per chunk
SHTOK = TOK // NCORES     # 1024 tokens per core for the S reduction
SHT = SHTOK // P          # 8 row-tiles of the S shard

_module_cache = {}
last_result = None


def _build_module():
    import concourse.mybir as mybir
    import concourse.tile as tile
    from concourse import bacc

    bf = mybir.dt.bfloat16
    f8 = mybir.dt.float8e4
    f32 = mybir.dt.float32
    AX = mybir.AxisListType
    DR = mybir.MatmulPerfMode.DoubleRow
    COPY = mybir.ActivationFunctionType.Copy

    nc = bacc.Bacc(None, target_bir_lowering=False, debug=False,
                   num_devices=NCORES)
    xt_d = nc.dram_tensor("xt", (NS, KKT, P, 2, MS), f8, kind="ExternalInput")
    wbl_d = nc.dram_tensor("wbl", (P, RG, 2, OC), bf, kind="ExternalInput")
    wbr_d = nc.dram_tensor("wbr", (P, RCH, RG, 2, RCW), f8, kind="ExternalInput")
    wbq_d = nc.dram_tensor("wbq", (P, KT, 2, OC), f8, kind="ExternalInput")
    XBH = D_IN // 2
    xb_d = nc.dram_tensor("xb", (SHT, P, 2, XBH), bf, kind="ExternalInput")
    bias_d = nc.dram_tensor("biasv", (P, OC), f32, kind="ExternalInput")
    y_d = nc.dram_tensor("y", (TOK, OC), f32, kind="ExternalOutput")

    with tile.TileContext(nc) as tc:
        with (
            tc.tile_pool(name="const", bufs=1) as const,
            tc.tile_pool(name="wpool", bufs=1) as wpool,
            tc.tile_pool(name="xpool", bufs=16) as xpool,
            tc.tile_pool(name="xbpool", bufs=8) as xbpool,
            tc.tile_pool(name="qpool", bufs=4) as qpool,
            tc.tile_pool(name="ypool", bufs=28) as ypool,
            tc.tile_pool(name="cpool", bufs=4) as cpool,
            tc.tile_pool(name="ppool", bufs=6, space="PSUM") as ppool,
            tc.tile_pool(name="wbpool", bufs=2, space="PSUM") as wbpool,
            tc.tile_pool(name="dpool", bufs=1, space="DRAM") as dpool,
        ):
            sin_d = dpool.tile([P, SHT], f32, name="sin")
            sout_d = dpool.tile([NCORES, P, SHT], f32, name="sout",
                                addr_space="Shared")
            WBL = const.tile([P, RG, 2, OC], bf)
            WBR = const.tile([P, RCH, RG, 2, RCW], f8)
            RD = const.tile([P, RCH, RG, RCW], bf)
            bias_t = const.tile([P, OC], f32)
            LdT = const.tile([P, RG, OC], bf)
            Wt = wpool.tile([P, KKT, 2, OC], f8)
            mneg = const.tile([P, RG], f32)     # -mean_i r_deq (per rank row)
            m16 = const.tile([P, RG], bf)       # +m (bf16, mu lhsT)
            Rpart = const.tile([P, RG, RCH], f32)
            ones1 = const.tile([1, P], bf)
            murow = const.tile([1, OC], bf)
            mub = const.tile([P, OC], f32)      # broadcast 16*mu/16 = mu
            S_sb = const.tile([P, NS * SUB], f32)
            Sown = const.tile([P, SHT], f32)
            Spart = const.tile([P, SHT, 2], f32)

            # ---- phase-0 DMAs.  sync/scalar: weights blob; gpsimd: x shard.
            nc.sync.dma_start(bias_t[:], bias_d[:])
            nc.sync.dma_start(WBL[:], wbl_d[:])
            for ch in range(RCH):
                eng = nc.sync if ch % 2 == 0 else nc.scalar
                eng.dma_start(WBR[:, ch], wbr_d[:, ch])
            xbt = []
            for j in range(SHT):
                for h in range(2):
                    t = xbpool.tile([P, XBH], bf, tag="xb", name=f"xb{j}_{h}")
                    nc.gpsimd.dma_start(t[:], xb_d[j, :, h])
                    xbt.append(t)

            # ---- dequantize L^T (codes x pre-broadcast scales/16)
            for j in range(RG):
                nc.vector.tensor_mul(LdT[:, j, :], WBL[:, j, 0, :],
                                     WBL[:, j, 1, :])
            nc.vector.memset(ones1[:], 1.0)

            # ---- dequantize R -> bf16, per-chunk -sum partials, mean, center
            for ch in range(RCH):
                for j in range(RG):
                    nc.vector.tensor_mul(RD[:, ch, j, :],
                                         WBR[:, ch, j, 0, :],
                                         WBR[:, ch, j, 1, :])
                    nc.vector.reduce_sum(Rpart[:, j, ch:ch + 1],
                                         RD[:, ch, j, :],
                                         axis=AX.X, negate=True)
            nc.vector.reduce_sum(mneg[:], Rpart[:], axis=AX.X)
            nc.vector.tensor_scalar_mul(m16[:], mneg[:], -1.0 / D_IN)
            nc.vector.tensor_scalar_mul(mneg[:], mneg[:], 1.0 / D_IN)
            for ch in range(RCH):
                for j in range(RG):
                    nc.vector.tensor_scalar_add(RD[:, ch, j, :],
                                                RD[:, ch, j, :],
                                                mneg[:, j:j + 1])

            # ---- mu = 16 * m @ (L/16)^T, broadcast to all 128 partitions
            mu_ps = wbpool.tile([P, OC], f32, tag="wb", name="mu1")
            for j in range(RG):
                nc.tensor.matmul(mu_ps[0:1, :], m16[:, j:j + 1], LdT[:, j, :],
                                 start=(j == 0), stop=(j == RG - 1))
            nc.vector.tensor_copy(murow[:], mu_ps[0:1, :])
            mub_ps = wbpool.tile([P, OC], f32, tag="wb", name="mu2")
            nc.tensor.matmul(mub_ps[:], ones1[:], murow[:], start=True,
                             stop=True)
            nc.vector.tensor_scalar_mul(mub[:], mub_ps[:], 16.0)

            # ---- W build: What k-tile k = R_hat^T(L/16)^T + Q^T/16 -> e4m3
            def rd(j, k):
                return RD[:, k // 4, j, (k % 4) * P:(k % 4) * P + P]

            def build_w(k):
                ps = wbpool.tile([P, OC], f32, tag="wb", name=f"wb{k}")
                for j in range(RG):
                    nc.tensor.matmul(ps[:], rd(j, k), LdT[:, j, :],
                                     start=(j == 0), stop=(j == RG - 1))
                qt = qpool.tile([P, 2, OC], f8, tag="qt")
                nc.sync.dma_start(qt[:], wbq_d[:, k])
                nc.vector.tensor_mul(qt[:, 0, :], qt[:, 0, :], qt[:, 1, :])
                nc.vector.tensor_add(Wt[:, k // 2, k % 2, :], ps[:],
                                     qt[:, 0, :])

            psums = {}

            def slab_mms(s, dma_engines):
                psums[s] = [ppool.tile([P, OC], f32, tag="ps",
                                       name=f"ps{s}_{i}") for i in range(SUB)]
                for kk in range(KKT):
                    xt = xpool.tile([P, 2, MS], f8, tag="x")
                    dma_engines[kk % len(dma_engines)].dma_start(
                        xt[:], xt_d[s, kk])
                    for sub in range(SUB):
                        nc.tensor.matmul(
                            psums[s][sub][:],
                            xt[:, :, sub * P:(sub + 1) * P],
                            Wt[:, kk, :, :],
                            start=(kk == 0), stop=(kk == KKT - 1),
                            perf_mode=DR,
                        )
                    yield kk

            ytiles = {}

            def evict_a(s):
                # psum + bias -> SBUF, frees the psum bank
                ytiles[s] = []
                for sub in range(SUB):
                    yt = ypool.tile([P, OC], f32, tag="y", name=f"y{s}_{sub}")
                    nc.vector.tensor_add(yt[:], psums[s][sub][:], bias_t[:])
                    ytiles[s].append(yt)

            def evict_b(s):
                # + S*mu (ACT outer product), then store
                for sub in range(SUB):
                    col = s * SUB + sub
                    corr = cpool.tile([P, OC], f32, tag="c")
                    nc.scalar.activation(corr[:], mub[:], COPY,
                                         scale=S_sb[:, col:col + 1])
                    yt = ytiles[s][sub]
                    nc.vector.tensor_add(yt[:], yt[:], corr[:])
                    eng = nc.scalar if sub % 2 == 0 else nc.sync
                    eng.dma_start(
                        y_d[s * MS + sub * P:s * MS + (sub + 1) * P, :], yt[:])

            # ---- slab 0 with the W build interleaved two tiles ahead
            for k in range(4):
                build_w(k)
            for kk in slab_mms(0, [nc.scalar]):
                if kk < KKT - 2:
                    build_w(2 * kk + 4)
                    build_w(2 * kk + 5)

            evict_a(0)

            # ---- S shard: reduce own 1024 bf16 token rows, AllGather 4 KB
            for i in range(2 * SHT):
                h = i % 2
                nc.vector.reduce_sum(Spart[:, i // 2, h:h + 1], xbt[i][:],
                                     axis=AX.X)
            nc.vector.reduce_sum(Sown[:], Spart[:], axis=AX.X)
            nc.gpsimd.dma_start(sin_d[:], Sown[:])
            nc.gpsimd.collective_compute(
                "AllGather",
                mybir.AluOpType.bypass,
                replica_groups=[list(range(NCORES))],
                ins=[sin_d.opt()],
                outs=[sout_d.opt()],
            )
            for c in range(NCORES):
                nc.gpsimd.dma_start(S_sb[:, c * SHT:(c + 1) * SHT], sout_d[c])

            # evict_b lags 5 slabs mid-run (rides out the S collective
            # latency), catches up two-per-slab from slab 10 so the tail
            # after the last matmul is a single slab.
            done_b = 0
            for s in range(1, NS):
                for kk in slab_mms(s, [nc.sync, nc.scalar]):
                    pass
                evict_a(s)
                want = s - 5 if s < 10 else 2 * (s - 10) + 6
                while done_b <= min(want, s - 1):
                    evict_b(done_b)
                    done_b += 1
            while done_b < NS:
                evict_b(done_b)
                done_b += 1

    nc.compile()
    return nc


def kernel(x, q_values, q_scales, l_values, l_scales, r_values, r_scales, bias,
           _trace=False):
    from concourse.bass_utils import run_bass_kernel_spmd

    bf16 = ml_dtypes.bfloat16
    e4m3 = ml_dtypes.float8_e4m3

    if "m" not in _module_cache:
        _module_cache["m"] = _build_module()
    nc = _module_cache["m"]

    # host-side marshaling (layout + dtype + power-of-two scaling only)
    x = np.asarray(x, dtype=np.float32)
    q_values = np.asarray(q_values)
    q_scales = np.asarray(q_scales, np.float32)
    l_values = np.asarray(l_values)
    l_scales = np.asarray(l_scales, np.float32)
    r_values = np.asarray(r_values)
    r_scales = np.asarray(r_scales, np.float32)
    bias = np.asarray(bias, np.float32)

    # x*16 as e4m3, tiled [NS, KKT, P, 2, MS]: plane ko holds i = kk*256+ko*128+p
    xs = np.clip(x * 16.0, -240.0, 240.0)
    xt8 = np.ascontiguousarray(
        xs.reshape(NS, MS, KKT, 2, P).transpose(0, 2, 4, 3, 1)
    ).astype(e4m3)
    # bf16 raw-row x for the S reduction, per-core shard below
    xb_all = x.astype(bf16)

    rs_full = np.repeat(r_scales, D_IN // r_scales.shape[1], axis=1)
    rv_f = r_values.astype(np.float32)
    # wbr[p, ch, j, 0, :] = r codes, [.., 1, :] = broadcast r scales
    wbr = np.empty((P, RCH, RG, 2, RCW), np.float32)
    for ch in range(RCH):
        cs = slice(ch * RCW, (ch + 1) * RCW)
        for j in range(RG):
            wbr[:, ch, j, 0, :] = rv_f[j * P:(j + 1) * P, cs]
            wbr[:, ch, j, 1, :] = rs_full[j * P:(j + 1) * P, cs]
    wbr = wbr.astype(e4m3)

    in_maps = []
    for c in range(NCORES):
        sl = slice(c * OC, (c + 1) * OC)
        qt_c = q_values[sl].T.astype(np.float32)            # [D_IN, OC]
        qst_c = (q_scales[sl].T / 16.0).astype(np.float32)  # [KT, OC]
        ltv_c = l_values[sl].T.astype(np.float32)           # [RANK, OC]
        lst_c = (l_scales[sl].T / 16.0).astype(np.float32)  # [RG, OC]

        wbl = np.empty((P, RG, 2, OC), np.float32)
        for j in range(RG):
            wbl[:, j, 0, :] = ltv_c[j * P:(j + 1) * P, :]
            wbl[:, j, 1, :] = np.broadcast_to(lst_c[j], (P, OC))
        wbq = np.empty((P, KT, 2, OC), np.float32)
        for k in range(KT):
            wbq[:, k, 0, :] = qt_c[k * P:(k + 1) * P, :]
            wbq[:, k, 1, :] = np.broadcast_to(qst_c[k], (P, OC))

        in_maps.append({
            "xt": xt8,
            "wbl": wbl.astype(bf16),
            "wbr": wbr,
            "wbq": wbq.astype(e4m3),
            "xb": np.ascontiguousarray(
                xb_all[c * SHTOK:(c + 1) * SHTOK].reshape(SHT, P, 2, D_IN // 2)),
            "biasv": np.ascontiguousarray(
                np.broadcast_to(bias[sl], (P, OC))).astype(np.float32),
        })

    res = run_bass_kernel_spmd(
        nc, in_maps, core_ids=list(range(NCORES)), trace=_trace
    )
    global last_result
    last_result = res
    return np.concatenate([r["y"] for r in res.results], axis=1)
